# revision 15
# baseline (speedup 1.0000x reference)
"""Trainium2 Bass kernel for nn_HandGNNEncoder (2-layer GCN on 21-node hand
graphs + mean pool), data-parallel over 8 NeuronCores.

Math restructure (exact):
  reference: h1 = relu(A @ (x @ W1) + b1); out = mean_t(A @ (h1 @ W2) + b2)
  mean-pool is linear, so with m[s] = column-mean of A (all > 0):
      out[g] = sum_s m[s] * h1[g,s,:] @ W2 + b2
  m[s] > 0 folds inside the relu:  m*relu(z) = relu(m*z).
  Stage 1 (PE): z[(s,f), g] = TW.T @ x'[g]   with TW[(s',c),(s,f)] =
      m[s]*A[s,s']*W1[c,f], bias row via a constant-1 input row, plus one
      extra column that relu's to the constant 1 (carries b2 in stage 2).
  Stage 2 (PE): out[d, g] = sum_k W2R_k.T @ relu_k  accumulated in PSUM.

Key implementation facts this version exploits:
  * Stage-2's k-tiles 0..9 all share the SAME stationary operand
    [[W2],[W2]] (the node-tiling redundancy), so the PE weight reload
    between those matmuls is redundant; they are emitted back-to-back.
  * Stage-1 matmuls alternate between PE row-halves (partitions 0-42 and
    64-106, with xt/tw duplicated on both halves) so each LDWEIGHTS
    targets row groups disjoint from the in-flight matmul and the PE's
    reorder window can pull it ahead (hides the 128-col ~107ns load).
  * The PSUM->SBUF relu drain is the throughput wall on TRN2 (matmul
    output must be fp32; DVE/ScalarE read 32b/lane/cycle).  Drains are
    one [128,1024] instruction per z-tile (2 PSUM banks) to amortize
    per-instruction overhead, split 6-on-DVE / 5-on-ScalarE.
  * PE warmup matmuls run during the input-DMA window so the HAM clock
    gate reaches 8/8 before real work, and a tiny early activation
    triggers the ScalarE table load off the critical path.
  * Output is DMA'd as f16 (cast host-side) to halve output traffic.
"""

import numpy as np

import concourse.bass as bass
import concourse.mybir as mybir
import concourse.tile as tile
from concourse import bass_utils

# ---- hardcoded problem constants ----
B, S, NNODE, CIN = 64, 512, 21, 2
D1, D2 = 64, 128
G = B * S                      # 32768 graphs
N_CORES = 8
G_CORE = G // N_CORES          # 4096 graphs per core
CHUNK = 1024                   # graphs per pipeline chunk
HALF = CHUNK // 2
N_CHUNKS = G_CORE // CHUNK
K1 = NNODE * CIN + 1           # 43 contraction rows (42 feats + ones row)
KT = 11                        # 1408 / 128 k-tiles for stage 2
M1 = KT * 128                  # 1408 = 1344 (s,f) cols + 1 bias col + 63 pad
N_PREWARM = 64                 # N=128 fillers bridging the input-DMA wait

EDGES = np.array(
    [[0, 1], [1, 2], [2, 3], [3, 4], [0, 5], [5, 6], [6, 7], [7, 8],
     [0, 9], [9, 10], [10, 11], [11, 12], [0, 13], [13, 14], [14, 15],
     [15, 16], [0, 17], [17, 18], [18, 19], [19, 20], [5, 9], [9, 13],
     [13, 17]], dtype=np.int64)


def fold_weights(W1, b1, W2, b2):
    """Fold adjacency, mean-pool and biases into dense operands.

    Returns (tw [43,1408], w2a [128,128], w2b [128,128]):
      tw    stage-1 stationary tiles (k-th tile = tw[:, 128k:128k+128])
      w2a   shared stage-2 stationary for k-tiles 0..9  ( [[W2],[W2]] )
      w2b   stage-2 stationary for k-tile 10 ( [W2; b2; zeros] )
    """
    W1 = np.asarray(W1, np.float32)
    b1 = np.asarray(b1, np.float32)
    W2 = np.asarray(W2, np.float32)
    b2 = np.asarray(b2, np.float32)
    A = np.eye(NNODE, dtype=np.float32)
    A[EDGES[:, 1], EDGES[:, 0]] = 1.0
    deg = A.sum(axis=1)
    dis = 1.0 / np.sqrt(deg)
    a_norm = dis[:, None] * A * dis[None, :]          # [t, s] float32
    m = a_norm.mean(axis=0)                           # [21], all > 0

    # tw[(s',c), (s,f)] = m[s] * a_norm[s, s'] * W1[c, f]
    tw = np.zeros((K1, M1), np.float32)
    blk = np.einsum("s,st,cf->tcsf", m, a_norm, W1)   # [s'=t, c, s, f]
    tw[: NNODE * CIN, : NNODE * D1] = blk.reshape(NNODE * CIN, NNODE * D1)
    tw[K1 - 1, : NNODE * D1] = (m[:, None] * b1[None, :]).reshape(-1)
    tw[K1 - 1, NNODE * D1] = 1.0                      # relu's to constant 1

    w2full = np.zeros((M1, D2), np.float32)
    w2full[: NNODE * D1] = np.tile(W2, (NNODE, 1))
    w2full[NNODE * D1] = b2                           # rides the const-1 row
    w2a = np.ascontiguousarray(w2full[:128])          # == [[W2],[W2]]
    w2b = np.ascontiguousarray(w2full[10 * 128:])     # [W2; b2; zeros]
    return tw, w2a, w2b


def build_wimg(tw, w2a, w2b, np_dt=np.float16):
    """Pack all stationary operands into one [128, 1024] SBUF image.

    cols 0..767   rows 0..42   : even k-tiles of tw (k = 0,2,...,10)
    cols 0..639   rows 64..106 : odd  k-tiles of tw (k = 1,3,...,9)
    cols 768..895 all rows     : w2a
    cols 896..1023 all rows    : w2b
    """
    img = np.zeros((128, 1024), np_dt)
    for j in range(6):                                # even k = 2j
        img[0:K1, 128 * j:128 * (j + 1)] = tw[:, 128 * (2 * j):128 * (2 * j + 1)]
    for j in range(5):                                # odd k = 2j+1
        img[64:64 + K1, 128 * j:128 * (j + 1)] = \
            tw[:, 128 * (2 * j + 1):128 * (2 * j + 2)]
    img[:, 768:896] = w2a
    img[:, 896:1024] = w2b
    return img


def s1_lhsT(wsb, k):
    if k % 2 == 0:
        return wsb[0:K1, 128 * (k // 2):128 * (k // 2) + 128]
    return wsb[64:64 + K1, 128 * ((k - 1) // 2):128 * ((k - 1) // 2) + 128]


def build_bass(op_dt="float16"):
    f16 = getattr(mybir.dt, op_dt)
    f32 = mybir.dt.float32
    nc = bass.Bass("TRN2", target_bir_lowering=False, debug=False)
    # xt is chunk-contiguous: chunk c occupies rows 43c..43c+42 so each
    # chunk load is one contiguous extent (fans out across SDMA engines).
    xt_d = nc.dram_tensor("xt", [K1 * N_CHUNKS, CHUNK], f16,
                          kind="ExternalInput").ap()
    w_d = nc.dram_tensor("wimg", [128, 1024], f16, kind="ExternalInput").ap()
    # out is chunk-contiguous too: chunk c at rows 128c..128c+127.
    out_d = nc.dram_tensor("out", [D2 * N_CHUNKS, CHUNK], f16,
                           kind="ExternalOutput").ap()

    relu = mybir.ActivationFunctionType.Relu
    copyf = mybir.ActivationFunctionType.Copy

    with tile.TileContext(nc) as tc:
        with (
            tc.tile_pool(name="w", bufs=1) as wpool,
            tc.tile_pool(name="r", bufs=2 * KT + 2) as rpool,
            tc.tile_pool(name="o", bufs=2) as opool,
            tc.tile_pool(name="pz", bufs=3, space="PSUM") as pzpool,
            tc.tile_pool(name="po", bufs=1, space="PSUM") as popool,
        ):
            # --- HAM warmers.  The PE clock gate is sticky: it only
            # un-throttles to 2.4 GHz after ~2 full 4096-cycle windows of
            # CONTINUOUS busy, and with a drain-paced pipeline that never
            # happens on its own.  Filler matmuls (garbage in, scratch
            # PSUM out) run before stage-1 and inside chunk-0's drain
            # stalls so the PE is continuously busy ~7us once, flipping
            # the gate for the whole run. ---
            scr = wpool.tile([128, HALF], f16, tag="scr")
            scrf = wpool.tile([1, 8], f16, tag="scrf")
            # memset on DVE so the gpsimd queue starts DMA descriptors
            # immediately; fillers must be FULL-K matmuls — the HAM
            # activity monitor watches MAC utilization, and a K=1 matmul
            # (1 of 128 rows) never registers as busy.
            nc.vector.memset(scr, 0.0)
            nc.scalar.activation(out=scrf[0:1, 0:8], in_=scr[0:1, 0:8],
                                 func=relu)
            warm_ps = popool.tile([D2, CHUNK], f32, tag="po", name="warm")

            def filler(n=128):
                # full-K, shared lhsT: full-array activity, LDW hidden
                nc.tensor.matmul(warm_ps[:, 0:n], lhsT=scr[:, 0:128],
                                 rhs=scr[:, 0:n], start=True, stop=True,
                                 skip_group_check=True)

            for _ in range(N_PREWARM):
                filler()

            # --- inputs.  Everything bulk goes via gpsimd SWDGE: each
            # dma_start fans out across all 16 SDMA engines (HWDGE pins a
            # start to ONE engine at ~25 GB/s), and descriptor generation
            # runs on the otherwise-idle GpSimd engine. ---
            wsb = wpool.tile([128, 1024], f16, tag="wsb")
            nc.gpsimd.dma_start(out=wsb, in_=w_d)
            w2a_sb = wsb[:, 768:896]
            w2b_sb = wsb[:, 896:1024]

            xt_sb = wpool.tile([107, G_CORE], f16, tag="xt")

            def load_xt(c):
                cs = CHUNK * c
                src = xt_d[K1 * c:K1 * (c + 1), :]
                nc.gpsimd.dma_start(out=xt_sb[0:43, cs:cs + CHUNK], in_=src)
                nc.gpsimd.dma_start(out=xt_sb[64:107, cs:cs + CHUNK],
                                    in_=src)

            for c in range(N_CHUNKS):
                load_xt(c)

            def s2_items(c, rts, out_ps):
                """The 22 stage-2 matmuls for chunk c as two
                self-contained half-blocks: H_h = [h k0..9 (shared w2a),
                w2b h].  Each half starts its PSUM accumulation group
                (start=True on k0) and closes it (stop=True on w2b), and
                the stationary operand changes only once per half."""
                items = []
                for h in (0, 1):
                    for k in range(KT - 1):
                        items.append((w2a_sb, rts[k], h, k == 0, False))
                    items.append((w2b_sb, rts[KT - 1], h, False, True))

                def emit(j):
                    lhsT, rt, h, st, sp = items[j]
                    nc.tensor.matmul(
                        out_ps[:, HALF * h:HALF * (h + 1)], lhsT=lhsT,
                        rhs=rt[:, HALF * h:HALF * (h + 1)],
                        start=st, stop=sp, skip_group_check=True)
                return emit

            def s2_finish(c, out_ps):
                # out copy (cast f32->f16): one [128,1024] op on DVE in its
                # stage-2 idle window; last chunk splits across SC/DVE so
                # the h0 half drains while h1 matmuls still run.
                ot = opool.tile([D2, CHUNK], f16, tag="o", name=f"os{c}")
                if c == N_CHUNKS - 1:
                    nc.scalar.activation(out=ot[:, 0:HALF],
                                         in_=out_ps[:, 0:HALF], func=copyf)
                    nc.vector.tensor_copy(out=ot[:, HALF:CHUNK],
                                          in_=out_ps[:, HALF:CHUNK])
                else:
                    nc.vector.tensor_copy(out=ot, in_=out_ps)
                nc.gpsimd.dma_start(out=out_d[D2 * c:D2 * (c + 1), :],
                                    in_=ot)

            # Per chunk c: stage-1 pairs with drains (5 on DVE, 6 on
            # ScalarE); the PREVIOUS chunk's stage-2 runs as two 11-MM
            # half-blocks placed after pairs k=5 and k=10, each short
            # enough that the 3 buffered z-tiles keep the drain engines
            # fed through it.
            prev = None            # (emit_fn, out_ps, c)
            for c in range(N_CHUNKS):
                cs = CHUNK * c
                rts = []
                for k in range(KT):
                    base = 0 if k % 2 == 0 else 64
                    zt = pzpool.tile([128, CHUNK], f32, tag="pz",
                                     name=f"z{c}_{k}")
                    for h in (0, 1):
                        nc.tensor.matmul(
                            zt[:, HALF * h:HALF * (h + 1)],
                            lhsT=s1_lhsT(wsb, k),
                            rhs=xt_sb[base:base + K1,
                                      cs + HALF * h:cs + HALF * (h + 1)],
                            start=True, stop=True, skip_group_check=True)
                    if c == 0 and k >= 1:
                        filler(HALF)
                        filler(HALF)
                    # drain: relu PSUM->SBUF f16, one [128,1024] op
                    rt = rpool.tile([128, CHUNK], f16, tag="r",
                                    name=f"r{c}_{k}")
                    if k in (0, 2, 4, 6, 8):
                        nc.vector.tensor_scalar_max(out=rt, in0=zt,
                                                    scalar1=0.0)
                    else:
                        nc.scalar.activation(out=rt, in_=zt, func=relu)
                    rts.append(rt)
                    if prev is not None and k in (5, 10):
                        emit, pps, pc = prev
                        if pc == 0:
                            # chunk-0's stage-2 runs as ONE 22-matmul block:
                            # ~9.4us of back-to-back cold matmuls is the one
                            # guaranteed way to flip the sticky HAM clock
                            # gate to 2.4 GHz for the rest of the run.
                            j0, j1 = (0, 0) if k == 5 else (0, 22)
                        else:
                            j0, j1 = (0, 11) if k == 5 else (11, 22)
                        for j in range(j0, j1):
                            emit(j)
                        if k == 10:
                            s2_finish(pc, pps)
                out_ps = popool.tile([D2, CHUNK], f32, tag="po",
                                     name=f"o{c}")
                prev = (s2_items(c, rts, out_ps), out_ps, c)
            emit, pps, pc = prev
            for j in range(22):
                emit(j)
            s2_finish(pc, pps)
    _rebalance_matmul_waits(nc)
    return nc


def _rebalance_matmul_waits(nc):
    """Walrus' TPB ISA structs accept only one sync-wait per instruction on
    the compute engines, but Tile can attach several (PE completion-order +
    cross-engine WAR + DMA). Keep one wait on the instruction and move the
    excess onto the immediately-preceding Ldweights (for matmuls) or onto
    freshly inserted same-engine NoOps — those execute just before on the
    same in-order queue, so waiting there is the same or stronger ordering."""
    import bass_rust
    import concourse.mybir as mybir

    exempt = {"InstEventSemaphore", "InstUnconditionalBranch",
              "InstCall", "InstISA", "InstNoOp"}
    nop_ctr = [0]
    for fn in nc.m.functions:
        for blk in fn.blocks:
            insts = list(blk.instructions)
            out = []
            pending_free_ldw = None
            for inst in insts:
                tn = type(inst).__name__
                if tn == "InstLdweights":
                    si = inst.sync_info
                    nw = len(si.on_wait) if si is not None else 0
                    if nw > 1:
                        for w in list(si.on_wait)[:-1]:
                            nop_ctr[0] += 1
                            nop = mybir.InstNoOp(
                                name=f"I-waitnop-{nop_ctr[0]}", ins=[],
                                outs=[])
                            nop.engine = inst.engine
                            nop.sync_info = bass_rust.SyncInfo(
                                on_wait=[w], on_update=[])
                            out.append(nop)
                        inst.sync_info = bass_rust.SyncInfo(
                            on_wait=list(si.on_wait)[-1:],
                            on_update=list(si.on_update))
                    elif nw == 0:
                        pending_free_ldw = inst
                    out.append(inst)
                    continue
                si = inst.sync_info
                nw = len(si.on_wait) if si is not None else 0
                if tn in exempt or nw <= 1:
                    out.append(inst)
                    if tn == "InstMatmult":
                        pending_free_ldw = None
                    continue
                waits = list(si.on_wait)
                moved, kept = waits[:-1], waits[-1:]
                if tn == "InstMatmult" and pending_free_ldw is not None \
                        and len(moved) == 1:
                    c = pending_free_ldw
                    csi = c.sync_info
                    c.sync_info = bass_rust.SyncInfo(
                        on_wait=moved,
                        on_update=list(csi.on_update) if csi else [])
                else:
                    for w in moved:
                        nop_ctr[0] += 1
                        nop = mybir.InstNoOp(
                            name=f"I-waitnop-{nop_ctr[0]}", ins=[], outs=[])
                        nop.engine = inst.engine
                        nop.sync_info = bass_rust.SyncInfo(
                            on_wait=[w], on_update=[])
                        out.append(nop)
                inst.sync_info = bass_rust.SyncInfo(
                    on_wait=kept, on_update=list(si.on_update))
                out.append(inst)
                if tn == "InstMatmult":
                    pending_free_ldw = None
            if len(out) != len(insts):
                blk.instructions = out


_NC_CACHE = None


def _get_nc():
    global _NC_CACHE
    if _NC_CACHE is None:
        _NC_CACHE = build_bass()
    return _NC_CACHE


def make_in_maps(hand_landmarks, W1, b1, W2, b2, np_dt=np.float16):
    tw, w2a, w2b = fold_weights(W1, b1, W2, b2)
    wimg = build_wimg(tw.astype(np_dt), w2a.astype(np_dt), w2b.astype(np_dt),
                      np_dt)
    x = np.asarray(hand_landmarks, np.float32).reshape(G, NNODE * CIN)
    xt = np.empty((K1, G), np_dt)
    xt[: NNODE * CIN] = x.T
    xt[K1 - 1] = 1.0
    maps = []
    for i in range(N_CORES):
        xc = xt[:, i * G_CORE:(i + 1) * G_CORE]
        # chunk-contiguous: rows 43c..43c+42 = chunk c
        xcc = np.ascontiguousarray(
            xc.reshape(K1, N_CHUNKS, CHUNK).transpose(1, 0, 2)
            .reshape(K1 * N_CHUNKS, CHUNK))
        maps.append({"xt": xcc, "wimg": wimg})
    return maps


def gather_out(results):
    # per-core out is [N_CHUNKS*128, CHUNK] chunk-contiguous -> [D2, G_CORE]
    cores = []
    for i in range(N_CORES):
        o = results[i]["out"].reshape(N_CHUNKS, D2, CHUNK)
        cores.append(o.transpose(1, 0, 2).reshape(D2, G_CORE))
    full = np.concatenate(cores, axis=1)
    return np.ascontiguousarray(full.astype(np.float32).T).reshape(B, S, D2)


def run(in_maps, trace=False, **kw):
    res = bass_utils.run_bass_kernel_spmd(
        _get_nc(), in_maps, core_ids=list(range(N_CORES)), trace=trace, **kw)
    return res


def kernel(hand_landmarks, W1, b1, W2, b2):
    in_maps = make_in_maps(hand_landmarks, W1, b1, W2, b2)
    res = run(in_maps)
    return gather_out(res.results)


# revision 18
# speedup vs baseline: 1.1561x; 1.1561x over previous
"""Trainium2 Bass kernel for nn_HandGNNEncoder (2-layer GCN on 21-node hand
graphs + mean pool), data-parallel over 8 NeuronCores.

Math restructure (exact):
  reference: h1 = relu(A @ (x @ W1) + b1); out = mean_t(A @ (h1 @ W2) + b2)
  mean-pool is linear, so with m[s] = column-mean of A (all > 0):
      out[g] = sum_s m[s] * h1[g,s,:] @ W2 + b2
  m[s] > 0 folds inside the relu:  m*relu(z) = relu(m*z).
  Stage 1 (PE): z[(s,f), g] = TW.T @ x'[g]   with TW[(s',c),(s,f)] =
      m[s]*A[s,s']*W1[c,f], bias row via a constant-1 input row, plus one
      extra column that relu's to the constant 1 (carries b2 in stage 2).
  Stage 2 (PE): out[d, g] = sum_k W2R_k.T @ relu_k  accumulated in PSUM.

Key implementation facts this version exploits:
  * Stage-2's k-tiles 0..9 all share the SAME stationary operand
    [[W2],[W2]] (the node-tiling redundancy), so the PE weight reload
    between those matmuls is redundant; they are emitted back-to-back.
  * Stage-1 matmuls alternate between PE row-halves (partitions 0-42 and
    64-106, with xt/tw duplicated on both halves) so each LDWEIGHTS
    targets row groups disjoint from the in-flight matmul and the PE's
    reorder window can pull it ahead (hides the 128-col ~107ns load).
  * The PSUM->SBUF relu drain is the throughput wall on TRN2 (matmul
    output must be fp32; DVE/ScalarE read 32b/lane/cycle).  Drains are
    one [128,1024] instruction per z-tile (2 PSUM banks) to amortize
    per-instruction overhead, split 6-on-DVE / 5-on-ScalarE.
  * PE warmup matmuls run during the input-DMA window so the HAM clock
    gate reaches 8/8 before real work, and a tiny early activation
    triggers the ScalarE table load off the critical path.
  * Output is DMA'd as f16 (cast host-side) to halve output traffic.
"""

import numpy as np

import concourse.bass as bass
import concourse.mybir as mybir
import concourse.tile as tile
from concourse import bass_utils

# ---- hardcoded problem constants ----
B, S, NNODE, CIN = 64, 512, 21, 2
D1, D2 = 64, 128
G = B * S                      # 32768 graphs
N_CORES = 8
G_CORE = G // N_CORES          # 4096 graphs per core
CHUNK = 512                    # graphs per compute chunk (1 PSUM bank)
LCHUNK = 1024                  # graphs per DMA load chunk
N_CHUNKS = G_CORE // CHUNK     # 8 compute chunks
N_LCHUNKS = G_CORE // LCHUNK   # 4 load chunks
K1 = NNODE * CIN + 1           # 43 contraction rows (42 feats + ones row)
KT = 11                        # 1408 / 128 k-tiles for stage 2
M1 = KT * 128                  # 1408 = 1344 (s,f) cols + 1 bias col + 63 pad
N_PREWARM = 64                 # N=128 fillers bridging the input-DMA wait

EDGES = np.array(
    [[0, 1], [1, 2], [2, 3], [3, 4], [0, 5], [5, 6], [6, 7], [7, 8],
     [0, 9], [9, 10], [10, 11], [11, 12], [0, 13], [13, 14], [14, 15],
     [15, 16], [0, 17], [17, 18], [18, 19], [19, 20], [5, 9], [9, 13],
     [13, 17]], dtype=np.int64)


def fold_weights(W1, b1, W2, b2):
    """Fold adjacency, mean-pool and biases into dense operands.

    Returns (tw [43,1408], w2a [128,128], w2b [128,128]):
      tw    stage-1 stationary tiles (k-th tile = tw[:, 128k:128k+128])
      w2a   shared stage-2 stationary for k-tiles 0..9  ( [[W2],[W2]] )
      w2b   stage-2 stationary for k-tile 10 ( [W2; b2; zeros] )
    """
    W1 = np.asarray(W1, np.float32)
    b1 = np.asarray(b1, np.float32)
    W2 = np.asarray(W2, np.float32)
    b2 = np.asarray(b2, np.float32)
    A = np.eye(NNODE, dtype=np.float32)
    A[EDGES[:, 1], EDGES[:, 0]] = 1.0
    deg = A.sum(axis=1)
    dis = 1.0 / np.sqrt(deg)
    a_norm = dis[:, None] * A * dis[None, :]          # [t, s] float32
    m = a_norm.mean(axis=0)                           # [21], all > 0

    # tw[(s',c), (s,f)] = m[s] * a_norm[s, s'] * W1[c, f]
    tw = np.zeros((K1, M1), np.float32)
    blk = np.einsum("s,st,cf->tcsf", m, a_norm, W1)   # [s'=t, c, s, f]
    tw[: NNODE * CIN, : NNODE * D1] = blk.reshape(NNODE * CIN, NNODE * D1)
    tw[K1 - 1, : NNODE * D1] = (m[:, None] * b1[None, :]).reshape(-1)
    tw[K1 - 1, NNODE * D1] = 1.0                      # relu's to constant 1

    w2full = np.zeros((M1, D2), np.float32)
    w2full[: NNODE * D1] = np.tile(W2, (NNODE, 1))
    w2full[NNODE * D1] = b2                           # rides the const-1 row
    w2a = np.ascontiguousarray(w2full[:128])          # == [[W2],[W2]]
    w2b = np.ascontiguousarray(w2full[10 * 128:])     # [W2; b2; zeros]
    return tw, w2a, w2b


def build_wimg(tw, w2a, w2b, np_dt=np.float16):
    """Pack all stationary operands into one [128, 1024] SBUF image.

    cols 0..767   rows 0..42   : even k-tiles of tw (k = 0,2,...,10)
    cols 0..639   rows 64..106 : odd  k-tiles of tw (k = 1,3,...,9)
    cols 768..895 all rows     : w2a
    cols 896..1023 all rows    : w2b
    """
    img = np.zeros((128, 1024), np_dt)
    for j in range(6):                                # even k = 2j
        img[0:K1, 128 * j:128 * (j + 1)] = tw[:, 128 * (2 * j):128 * (2 * j + 1)]
    for j in range(5):                                # odd k = 2j+1
        img[64:64 + K1, 128 * j:128 * (j + 1)] = \
            tw[:, 128 * (2 * j + 1):128 * (2 * j + 2)]
    img[:, 768:896] = w2a
    img[:, 896:1024] = w2b
    return img


def s1_lhsT(wsb, k):
    if k % 2 == 0:
        return wsb[0:K1, 128 * (k // 2):128 * (k // 2) + 128]
    return wsb[64:64 + K1, 128 * ((k - 1) // 2):128 * ((k - 1) // 2) + 128]


def build_bass(op_dt="float16"):
    f16 = getattr(mybir.dt, op_dt)
    f32 = mybir.dt.float32
    nc = bass.Bass("TRN2", target_bir_lowering=False, debug=False)
    # xt is chunk-contiguous: chunk c occupies rows 43c..43c+42 so each
    # chunk load is one contiguous extent (fans out across SDMA engines).
    xt_d = nc.dram_tensor("xt", [K1 * N_LCHUNKS, LCHUNK], f16,
                          kind="ExternalInput").ap()
    w_d = nc.dram_tensor("wimg", [128, 1024], f16, kind="ExternalInput").ap()
    # out is chunk-contiguous too: chunk c at rows 128c..128c+127.
    out_d = nc.dram_tensor("out", [D2 * N_LCHUNKS, LCHUNK], f16,
                           kind="ExternalOutput").ap()

    relu = mybir.ActivationFunctionType.Relu
    copyf = mybir.ActivationFunctionType.Copy

    with tile.TileContext(nc) as tc:
        with (
            tc.tile_pool(name="w", bufs=1) as wpool,
            tc.tile_pool(name="r", bufs=3 * KT + 1) as rpool,
            tc.tile_pool(name="o", bufs=2) as opool,
            tc.tile_pool(name="pz", bufs=6, space="PSUM") as pzpool,
            tc.tile_pool(name="po", bufs=2, space="PSUM") as popool,
        ):
            # --- HAM warmers.  The PE clock gate is sticky: it only
            # un-throttles to 2.4 GHz after ~2 full 4096-cycle windows of
            # CONTINUOUS busy, and with a drain-paced pipeline that never
            # happens on its own.  Filler matmuls (garbage in, scratch
            # PSUM out) run before stage-1 and inside chunk-0's drain
            # stalls so the PE is continuously busy ~7us once, flipping
            # the gate for the whole run. ---
            scr = wpool.tile([128, CHUNK], f16, tag="scr")
            scrf = wpool.tile([1, 8], f16, tag="scrf")
            # memset on DVE so the gpsimd queue starts DMA descriptors
            # immediately; fillers must be FULL-K matmuls — the HAM
            # activity monitor watches MAC utilization, and a K=1 matmul
            # (1 of 128 rows) never registers as busy.
            nc.vector.memset(scr, 0.0)
            nc.scalar.activation(out=scrf[0:1, 0:8], in_=scr[0:1, 0:8],
                                 func=relu)
            warm_ps = popool.tile([D2, CHUNK], f32, tag="po", name="warm")

            def filler(n=128):
                # full-K, shared lhsT: full-array activity, LDW hidden
                nc.tensor.matmul(warm_ps[:, 0:n], lhsT=scr[:, 0:128],
                                 rhs=scr[:, 0:n], start=True, stop=True,
                                 skip_group_check=True)

            for _ in range(N_PREWARM):
                filler()

            # --- inputs.  Everything bulk goes via gpsimd SWDGE: each
            # dma_start fans out across all 16 SDMA engines (HWDGE pins a
            # start to ONE engine at ~25 GB/s), and descriptor generation
            # runs on the otherwise-idle GpSimd engine. ---
            wsb = wpool.tile([128, 1024], f16, tag="wsb")
            nc.gpsimd.dma_start(out=wsb, in_=w_d)
            w2a_sb = wsb[:, 768:896]
            w2b_sb = wsb[:, 896:1024]

            xt_sb = wpool.tile([107, G_CORE], f16, tag="xt")

            def load_xt(L):
                cs = LCHUNK * L
                src_ = xt_d[K1 * L:K1 * (L + 1), :]
                nc.gpsimd.dma_start(out=xt_sb[0:43, cs:cs + LCHUNK], in_=src_)
                nc.gpsimd.dma_start(out=xt_sb[64:107, cs:cs + LCHUNK],
                                    in_=src_)

            load_xt(0)
            load_xt(1)

            # ---- per-chunk emitters -------------------------------------
            ots = {}

            def s1(c):
                cs = CHUNK * c
                rts = []
                for k in range(KT):
                    base = 0 if k % 2 == 0 else 64
                    zt = pzpool.tile([128, CHUNK], f32, tag="pz",
                                     name=f"z{c}_{k}")
                    nc.tensor.matmul(
                        zt, lhsT=s1_lhsT(wsb, k),
                        rhs=xt_sb[base:base + K1, cs:cs + CHUNK],
                        start=True, stop=True, skip_group_check=True)
                    # drain: relu PSUM->SBUF f16; 6 on DVE, 5 on ScalarE
                    rt = rpool.tile([128, CHUNK], f16, tag="r",
                                    name=f"r{c}_{k}")
                    if k % 2 == 0:
                        nc.vector.tensor_scalar_max(out=rt, in0=zt,
                                                    scalar1=0.0)
                    else:
                        nc.scalar.activation(out=rt, in_=zt, func=relu)
                    rts.append(rt)
                return rts

            def s2(c, rts):
                # 11 matmuls sharing stationaries (w2a k0..9, then w2b),
                # accumulated into one PSUM bank; then the f32->f16 copy.
                out_ps = popool.tile([D2, CHUNK], f32, tag="po",
                                     name=f"o{c}")
                for k in range(KT - 1):
                    nc.tensor.matmul(out_ps, lhsT=w2a_sb, rhs=rts[k],
                                     start=(k == 0), stop=False,
                                     skip_group_check=True)
                nc.tensor.matmul(out_ps, lhsT=w2b_sb, rhs=rts[KT - 1],
                                 start=False, stop=True,
                                 skip_group_check=True)
                L, h = divmod(c, 2)
                if h == 0:
                    ots[L] = opool.tile([D2, LCHUNK], f16, tag="o",
                                        name=f"os{L}")
                ot = ots[L]
                dst = ot[:, CHUNK * h:CHUNK * (h + 1)]
                if c % 2 == 0:
                    nc.scalar.activation(out=dst, in_=out_ps, func=copyf)
                else:
                    nc.vector.tensor_copy(out=dst, in_=out_ps)
                if h == 1:
                    nc.gpsimd.dma_start(
                        out=out_d[D2 * L:D2 * (L + 1), :], in_=ot)

            # ---- schedule.  s2(c) is emitted after s1(c+1); chunks 0+1's
            # stage-2 run as ONE merged 22-matmul block after s1(2) — at
            # 1.2 GHz that is ~9.4us of gapless full-K matmuls, the one
            # guaranteed way to flip the sticky HAM clock gate, after
            # which the steady pipeline keeps it at 2.4 GHz. ----
            rts_live = {}
            for c in range(N_CHUNKS):
                if c % 2 == 0 and c // 2 + 2 < N_LCHUNKS:
                    load_xt(c // 2 + 2)
                rts_live[c] = s1(c)
                if c == 2:
                    s2(0, rts_live.pop(0))
                    s2(1, rts_live.pop(1))
                elif c >= 3:
                    s2(c - 1, rts_live.pop(c - 1))
            s2(N_CHUNKS - 1, rts_live.pop(N_CHUNKS - 1))
    _rebalance_matmul_waits(nc)
    return nc


def _rebalance_matmul_waits(nc):
    """Walrus' TPB ISA structs accept only one sync-wait per instruction on
    the compute engines, but Tile can attach several (PE completion-order +
    cross-engine WAR + DMA). Keep one wait on the instruction and move the
    excess onto the immediately-preceding Ldweights (for matmuls) or onto
    freshly inserted same-engine NoOps — those execute just before on the
    same in-order queue, so waiting there is the same or stronger ordering."""
    import bass_rust
    import concourse.mybir as mybir

    exempt = {"InstEventSemaphore", "InstUnconditionalBranch",
              "InstCall", "InstISA", "InstNoOp"}
    nop_ctr = [0]
    for fn in nc.m.functions:
        for blk in fn.blocks:
            insts = list(blk.instructions)
            out = []
            pending_free_ldw = None
            for inst in insts:
                tn = type(inst).__name__
                if tn == "InstLdweights":
                    si = inst.sync_info
                    nw = len(si.on_wait) if si is not None else 0
                    if nw > 1:
                        for w in list(si.on_wait)[:-1]:
                            nop_ctr[0] += 1
                            nop = mybir.InstNoOp(
                                name=f"I-waitnop-{nop_ctr[0]}", ins=[],
                                outs=[])
                            nop.engine = inst.engine
                            nop.sync_info = bass_rust.SyncInfo(
                                on_wait=[w], on_update=[])
                            out.append(nop)
                        inst.sync_info = bass_rust.SyncInfo(
                            on_wait=list(si.on_wait)[-1:],
                            on_update=list(si.on_update))
                    elif nw == 0:
                        pending_free_ldw = inst
                    out.append(inst)
                    continue
                si = inst.sync_info
                nw = len(si.on_wait) if si is not None else 0
                if tn in exempt or nw <= 1:
                    out.append(inst)
                    if tn == "InstMatmult":
                        pending_free_ldw = None
                    continue
                waits = list(si.on_wait)
                moved, kept = waits[:-1], waits[-1:]
                if tn == "InstMatmult" and pending_free_ldw is not None \
                        and len(moved) == 1:
                    c = pending_free_ldw
                    csi = c.sync_info
                    c.sync_info = bass_rust.SyncInfo(
                        on_wait=moved,
                        on_update=list(csi.on_update) if csi else [])
                else:
                    for w in moved:
                        nop_ctr[0] += 1
                        nop = mybir.InstNoOp(
                            name=f"I-waitnop-{nop_ctr[0]}", ins=[], outs=[])
                        nop.engine = inst.engine
                        nop.sync_info = bass_rust.SyncInfo(
                            on_wait=[w], on_update=[])
                        out.append(nop)
                inst.sync_info = bass_rust.SyncInfo(
                    on_wait=kept, on_update=list(si.on_update))
                out.append(inst)
                if tn == "InstMatmult":
                    pending_free_ldw = None
            if len(out) != len(insts):
                blk.instructions = out


_NC_CACHE = None


def _get_nc():
    global _NC_CACHE
    if _NC_CACHE is None:
        _NC_CACHE = build_bass()
    return _NC_CACHE


def make_in_maps(hand_landmarks, W1, b1, W2, b2, np_dt=np.float16):
    tw, w2a, w2b = fold_weights(W1, b1, W2, b2)
    wimg = build_wimg(tw.astype(np_dt), w2a.astype(np_dt), w2b.astype(np_dt),
                      np_dt)
    x = np.asarray(hand_landmarks, np.float32).reshape(G, NNODE * CIN)
    xt = np.empty((K1, G), np_dt)
    xt[: NNODE * CIN] = x.T
    xt[K1 - 1] = 1.0
    maps = []
    for i in range(N_CORES):
        xc = xt[:, i * G_CORE:(i + 1) * G_CORE]
        # chunk-contiguous: rows 43c..43c+42 = chunk c
        xcc = np.ascontiguousarray(
            xc.reshape(K1, N_LCHUNKS, LCHUNK).transpose(1, 0, 2)
            .reshape(K1 * N_LCHUNKS, LCHUNK))
        maps.append({"xt": xcc, "wimg": wimg})
    return maps


def gather_out(results):
    # per-core out is [N_CHUNKS*128, CHUNK] chunk-contiguous -> [D2, G_CORE]
    cores = []
    for i in range(N_CORES):
        o = results[i]["out"].reshape(N_LCHUNKS, D2, LCHUNK)
        cores.append(o.transpose(1, 0, 2).reshape(D2, G_CORE))
    full = np.concatenate(cores, axis=1)
    return np.ascontiguousarray(full.astype(np.float32).T).reshape(B, S, D2)


def run(in_maps, trace=False, **kw):
    res = bass_utils.run_bass_kernel_spmd(
        _get_nc(), in_maps, core_ids=list(range(N_CORES)), trace=trace, **kw)
    return res


def kernel(hand_landmarks, W1, b1, W2, b2):
    in_maps = make_in_maps(hand_landmarks, W1, b1, W2, b2)
    res = run(in_maps)
    return gather_out(res.results)
